# revision 15
# baseline (speedup 1.0000x reference)
"""Trainium2 Bass kernel for nn_DetectSpikes (spatiotemporal NMS spike detection).

kernel(traces [150000,384] f32, channel_locations [384,2] f32) ->
(times int64 [100000], chans int32 [100000]) matching the reference exactly.

Detection rule (in x_inv = -traces): (n, m) is a detection iff
x_inv >= 3.0, time margin, and x_inv >= max over adj(m) x [n-15, n+15]
(self included; ties pass).

Device (8 cores, time-sharded with halo, SPMD):
  - PE transposes [time, chan] DRAM tiles into [chan, time] SBUF layout with
    a fused negation (matmul against -I).
  - 8-sample block maxima B8 per channel (VE reduce).
  - Screen: two offset 16-sample grids; the grid block containing n always
    lies inside n's +-15 window, so a lower bound of the covered
    neighborhood max screens IN every true detection.  The cross-channel
    max is lower-bounded by log-sum-exp: w = exp(C*(B - 3)) (ACT), one
    small adjacency matmul per block (PE, bf16, exact-direction guarded),
    ln (ACT), then a single full-resolution compare produces a sparse u8
    superset mask (~67k of 57.6M).
  - Tables for host resolution: c8 = exact 5x8-block cover max per channel;
    Zc = sum-exp over adjacency-minus-self of c8.
Host: sparse nonzero, classify each screen point as sure-valid
  (x >= c8 own-cover max and x >= LSE upper bound from Zc) or ambiguous
  (~16k); resolve ambiguous exactly from raw traces (vectorized, pruned per
  neighbor by c8).  Output is exact.
"""

import time

import numpy as np

import concourse.bass as bass
import concourse.tile as tile
from concourse import bacc, mybir
from concourse.bass_utils import run_bass_kernel_spmd

import ml_dtypes

# ---- problem constants ----
N, M = 150000, 384
TR = 15
THR = 3.0
MARGIN = 20
RADIUS = 100.0
MAX_DET = 100000
NCORES = 8
INT = N // NCORES             # 18750

CHUNK = 512
NCHUNK_FULL = 37              # 37*512 = 18944 >= INT + 2*TR
T_LOC = NCHUNK_FULL * CHUNK
NEG_BIG = -3.0e38

S_SCALE = 32.0
C_LN = float(S_SCALE * np.log(2.0))
GUARD = 0.05
GUARD_SURE = 0.01
D_TOTAL_MAX = 64.0
B0 = 5.4      # exp-domain shift so ln inputs stay within +-2^64
CL = 1.7      # low clamp on decimated fields (< THR - slack, so decision-free)

_F32 = mybir.dt.float32
_BF16 = mybir.dt.bfloat16
_U8 = mybir.dt.uint8


def _strided(ap_tile, col, step, count):
    """[128, count] AP over tile columns col, col+step, ... (free stride)."""
    full = ap_tile[:]
    return bass.AP(full.tensor, full.offset + col,
                   [list(full.ap[0]), [step, count]])


def _bcast(ap_tile, cols, rep):
    """[128, cols, rep] AP with step-0 inner dim over tile[:, :cols]."""
    full = ap_tile[:]
    return bass.AP(full.tensor, full.offset,
                   [list(full.ap[0]), [1, cols], [0, rep]])


def build_program(n_chunks=NCHUNK_FULL):
    t_loc = n_chunks * CHUNK
    nb8 = t_loc // 8
    nc = bacc.Bacc(
        "TRN2", target_bir_lowering=False, debug=False, enable_asserts=False,
        num_devices=NCORES,
    )
    xs = nc.dram_tensor("xs", [t_loc, 384], _F32, kind="ExternalInput")
    negi = nc.dram_tensor("negi", [128, 128], _F32, kind="ExternalInput")
    wadj = nc.dram_tensor("wadj", [3, 3, 128, 128], _BF16, kind="ExternalInput")
    mask_d = nc.dram_tensor("mask", [3, 128, t_loc], _U8, kind="ExternalOutput")
    c8_d = nc.dram_tensor("c8", [3, 128, nb8], _F32, kind="ExternalOutput")

    BEXP = float(np.float32(-C_LN * B0))

    from contextlib import ExitStack
    with tile.TileContext(nc) as tc, ExitStack() as ctx:
        consts = ctx.enter_context(tc.tile_pool(name="consts", bufs=1))
        persist = ctx.enter_context(tc.tile_pool(name="persist", bufs=1))
        rawp = ctx.enter_context(tc.tile_pool(name="raw", bufs=3))
        xinp = ctx.enter_context(tc.tile_pool(name="xin", bufs=3))
        psx = ctx.enter_context(tc.tile_pool(name="psx", bufs=1, space="PSUM"))
        psz = ctx.enter_context(tc.tile_pool(name="psz", bufs=1, space="PSUM"))
        smallp = ctx.enter_context(tc.tile_pool(name="small", bufs=3))
        maskp = ctx.enter_context(tc.tile_pool(name="maskp", bufs=3))

        negi_t = consts.tile([128, 128], _F32, tag="negi")
        nc.sync.dma_start(negi_t[:], negi.ap()[:, :])
        bexp_t = consts.tile([128, 1], _F32, tag="bexp")
        nc.vector.memset(bexp_t[:], BEXP)
        thr3_t = consts.tile([128, 1], _F32, tag="thr3")
        nc.vector.memset(thr3_t[:], THR)
        e3_t = consts.tile([128, 1], _F32, tag="e3")
        e3s_t = consts.tile([128, 1], _F32, tag="e3s")
        wt = {}
        for sb in range(3):
            for db in range(3):
                a = consts.tile([128, 128], _BF16, tag=f"wa{sb}{db}")
                nc.sync.dma_start(a[:], wadj.ap()[sb, db, :, :])
                wt[("a", sb, db)] = a

        nc.scalar.activation(e3_t[:], thr3_t[:],
                             mybir.ActivationFunctionType.Exp,
                             bias=bexp_t[:], scale=C_LN)
        nc.vector.tensor_scalar(e3s_t[:], e3_t[:], 64.0, None,
                                mybir.AluOpType.mult)

        b8w = nb8 + 4
        B8g = []
        for db in range(3):
            t = persist.tile([128, b8w], _F32, tag=f"b8g{db}")
            nc.vector.memset(t[:, 0:2], NEG_BIG)
            nc.vector.memset(t[:, b8w - 2 : b8w], NEG_BIG)
            B8g.append(t)

        def b8c(k):
            return k + 2

        xin_hist = {}
        aux = {"w0": {}, "w1": {}}

        xs_r = xs.ap().rearrange("(i j p) c -> i p j c", p=128, j=4)

        def phase_load(i):
            raw = rawp.tile([128, 4 * 384], _F32, tag="raw")
            raw_v = raw[:].rearrange("p (j c) -> p j c", c=384)
            nc.sync.dma_start(raw_v, xs_r[i])
            xin = []
            for db in range(3):
                ps = psx.tile([128, CHUNK], _F32, tag=f"psx{db}")
                for j in range(4):
                    nc.tensor.matmul(
                        ps[:, 128 * j : 128 * (j + 1)],
                        raw_v[:, j, 128 * db : 128 * (db + 1)],
                        negi_t[:],
                        is_transpose=True,
                        start=True, stop=True,
                    )
                xt = xinp.tile([128, CHUNK], _F32, tag=f"xin{db}")
                nc.scalar.mul(xt[:], ps[:], -1.0)
                et = xinp.tile([128, CHUNK], _F32, tag=f"ex{db}")
                nc.scalar.activation(et[:], xt[:],
                                     mybir.ActivationFunctionType.Exp,
                                     bias=bexp_t[:], scale=C_LN)
                xin.append((xt, et))
                nc.vector.tensor_reduce(
                    B8g[db][:, b8c(64 * i) : b8c(64 * i) + 64],
                    xt[:].rearrange("p (a b) -> p a b", b=8),
                    axis=mybir.AxisListType.X,
                    op=mybir.AluOpType.max,
                )
            xin_hist[i] = xin

        def phase_tables(ii):
            base = 64 * ii
            for sb in range(3):
                B8 = B8g[sb]
                # sure-side: c8[k] = max(B8[k-2..k+2]), k in [base, base+64)
                P2 = smallp.tile([128, 66], _F32, tag="p2")
                nc.vector.tensor_tensor(
                    P2[:],
                    B8[:, b8c(base - 2) : b8c(base - 2) + 66],
                    B8[:, b8c(base - 1) : b8c(base - 1) + 66],
                    mybir.AluOpType.max,
                )
                t1 = smallp.tile([128, 64], _F32, tag="t1")
                nc.vector.tensor_tensor(
                    t1[:], P2[:, 0:64], P2[:, 2:66], mybir.AluOpType.max
                )
                c8t = smallp.tile([128, 64], _F32, tag="c8t")
                nc.vector.scalar_tensor_tensor(
                    c8t[:], t1[:], CL,
                    B8[:, b8c(base + 2) : b8c(base + 2) + 64],
                    mybir.AluOpType.max, mybir.AluOpType.max,
                )
                nc.sync.dma_start(c8_d.ap()[sb, :, base : base + 64], c8t[:])

                # screen-side grids
                g0 = smallp.tile([128, 32], _F32, tag="g0")
                nc.vector.scalar_tensor_tensor(
                    g0[:],
                    _strided(B8, b8c(base), 2, 32), CL,
                    _strided(B8, b8c(base + 1), 2, 32),
                    mybir.AluOpType.max, mybir.AluOpType.max,
                )
                g1 = smallp.tile([128, 33], _F32, tag="g1")
                nc.vector.scalar_tensor_tensor(
                    g1[:],
                    _strided(B8, b8c(base - 1), 2, 33), CL,
                    _strided(B8, b8c(base), 2, 33),
                    mybir.AluOpType.max, mybir.AluOpType.max,
                )
                w0 = smallp.tile([128, 32], _BF16, tag=f"w0{sb}")
                nc.scalar.activation(
                    w0[:], g0[:], mybir.ActivationFunctionType.Exp,
                    bias=bexp_t[:], scale=C_LN,
                )
                w1 = smallp.tile([128, 33], _BF16, tag=f"w1{sb}")
                nc.scalar.activation(
                    w1[:], g1[:], mybir.ActivationFunctionType.Exp,
                    bias=bexp_t[:], scale=C_LN,
                )
                aux["w0"][(ii, sb)] = w0
                aux["w1"][(ii, sb)] = w1

            for db in range(3):
                z0 = psz.tile([128, 32], _F32, tag="z0")
                z1 = psz.tile([128, 33], _F32, tag="z1")
                for sb in range(3):
                    nc.tensor.matmul(
                        z0[:], wt[("a", sb, db)][:], aux["w0"][(ii, sb)][:],
                        start=(sb == 0), stop=(sb == 2),
                    )
                    nc.tensor.matmul(
                        z1[:], wt[("a", sb, db)][:], aux["w1"][(ii, sb)][:],
                        start=(sb == 0), stop=(sb == 2),
                    )
                z1s = smallp.tile([128, 33], _F32, tag="z1s")
                nc.scalar.copy(z1s[:], z1[:])
                # Zs[k8] = Z0[k8//2] + Z1[(k8-1)//2]
                zs = smallp.tile([128, 64], _F32, tag="zs")
                in0 = bass.AP(z0[:].tensor, z0[:].offset,
                              [list(z0[:].ap[0]), [1, 32], [0, 2]])
                in1 = bass.AP(z1s[:].tensor, z1s[:].offset,
                              [list(z1s[:].ap[0]), [1, 32], [1, 2]])
                nc.vector.tensor_tensor(
                    zs[:].rearrange("p (a b) -> p a b", b=2), in0, in1,
                    mybir.AluOpType.add,
                )
                # fold threshold: zsk = max(Zs, 64*hwexp(C*(3-B0)))
                zsk = smallp.tile([128, 64], _F32, tag="zsk")
                e3b = bass.AP(e3s_t[:].tensor, e3s_t[:].offset,
                              [list(e3s_t[:].ap[0]), [0, 64]])
                nc.vector.tensor_tensor(
                    zsk[:], zs[:], e3b, mybir.AluOpType.max,
                )
                # screen: 64*E >= zsk  (table-independent superset)
                mt = maskp.tile([128, CHUNK], _U8, tag="mask")
                et = xin_hist[ii][db][1]
                nc.vector.scalar_tensor_tensor(
                    mt[:].rearrange("p (a b) -> p a b", b=8),
                    et[:].rearrange("p (a b) -> p a b", b=8),
                    64.0,
                    _bcast(zsk, 64, 8),
                    mybir.AluOpType.mult,
                    mybir.AluOpType.is_ge,
                )
                nc.sync.dma_start(
                    mask_d.ap()[db, :, CHUNK * ii : CHUNK * (ii + 1)], mt[:]
                )

        for i in range(n_chunks + 1):
            if i < n_chunks:
                phase_load(i)
            if i >= 1:
                ii = i - 1
                phase_tables(ii)
                xin_hist.pop(ii, None)
                for k in ("w0", "w1"):
                    for sb in range(3):
                        aux[k].pop((ii, sb), None)

    nc.compile()
    return nc


# ------------------------ host side ------------------------

def _adjacency(channel_locations):
    locs = np.asarray(channel_locations, np.float32)
    d2 = ((locs[:, None, :] - locs[None, :, :]) ** 2).sum(-1)
    return d2 <= np.float32(RADIUS) ** 2


def _const_inputs(adj):
    adj_f = adj.astype(np.float32)
    a = adj_f.reshape(3, 128, 3, 128).transpose(0, 2, 1, 3)
    a = np.ascontiguousarray(a).astype(ml_dtypes.bfloat16)
    negi = np.eye(128, dtype=np.float32)
    return negi, a


def _nbr_table(adj):
    deg = adj.sum(0)
    dmax = int(deg.max())
    nbr = np.zeros((M, dmax), np.int32)
    for m in range(M):
        js = np.flatnonzero(adj[:, m])
        nbr[m, : len(js)] = js
        nbr[m, len(js):] = js[0] if len(js) else m
    return nbr


def _nonzero_u8(mask2d):
    flat = np.ascontiguousarray(mask2d).reshape(-1)
    pad = (-flat.size) % 8
    if pad:
        flat = np.concatenate([flat, np.zeros(pad, np.uint8)])
    words = flat.view(np.uint64)
    wnz = np.flatnonzero(words)
    if wnz.size == 0:
        return (np.empty(0, np.int64), np.empty(0, np.int64))
    cand = (wnz[:, None] * 8 + np.arange(8)[None, :]).reshape(-1)
    cand = cand[cand < mask2d.size]
    cand = cand[flat[cand] != 0]
    T = mask2d.shape[1]
    return cand // T, cand % T


def _postprocess_core(mask, c8, traces, nbr, o_c, g0_row, n_int, n_glob):
    """mask [384, T_loc] u8, c8 [384, nb8] f32; traces full [n_glob, 384].
    o_c: local offset of interior start; g0_row: global row of interior start.
    Returns (times, chans) sorted by (t, chan) for this core's interior."""
    mm, lt = _nonzero_u8(mask[:, o_c : o_c + n_int])
    tg = lt + g0_row
    xv = -traces[tg, mm]
    keep = (tg >= MARGIN) & (tg < n_glob - MARGIN) & (xv >= THR)
    mm, tg, xv = mm[keep], tg[keep], xv[keep]
    if mm.size == 0:
        return np.empty(0, np.int64), np.empty(0, np.int64)
    ltf = tg - g0_row + o_c
    k8 = ltf // 8
    # exact sure-check: x beats every neighbor's 5x8-block cover max
    cwn = nbr[mm]                                      # [P, D]
    c8n = c8[cwn, k8[:, None]]                         # [P, D]
    m1 = c8n.max(1)
    sure = xv >= m1
    ok = sure.copy()
    amb = np.flatnonzero(~sure)
    if amb.size:
        tga = tg[amb]
        xva = xv[amb]
        cwa = cwn[amb]
        live = c8n[amb] >= xva[:, None]
        pi, di = np.nonzero(live)
        bad = np.zeros(amb.size, bool)
        if pi.size:
            tt = tga[pi]
            jj = cwa[pi, di]
            t0 = np.maximum(tt - TR, 0)
            t1 = np.minimum(tt + TR, n_glob - 1)
            tw = t0[:, None] + np.arange(2 * TR + 1)[None, :]
            np.minimum(tw, t1[:, None], out=tw)
            g = traces[tw, jj[:, None]]
            svp = -(g.min(1))
            veto = svp > xva[pi]
            bad = np.bincount(pi, weights=veto.astype(np.float64),
                              minlength=amb.size) > 0
        ok[amb] = ~bad
    mm, tg = mm[ok], tg[ok]
    o = np.lexsort((mm, tg))
    return tg[o], mm[o]


_PROGRAM_CACHE = {}


def kernel(traces, channel_locations):
    traces = np.ascontiguousarray(np.asarray(traces, np.float32))
    adj = _adjacency(channel_locations)
    negi, wa = _const_inputs(adj)
    nbr = _nbr_table(adj)

    if "full" not in _PROGRAM_CACHE:
        _PROGRAM_CACHE["full"] = build_program(NCHUNK_FULL)
    nc = _PROGRAM_CACHE["full"]

    starts = [min(max(c * INT - TR, 0), N - T_LOC) for c in range(NCORES)]
    in_maps = [{
        "xs": traces[starts[c] : starts[c] + T_LOC],
        "negi": negi,
        "wadj": wa,
    } for c in range(NCORES)]
    try:
        res = run_bass_kernel_spmd(nc, in_maps, list(range(NCORES)))
    except Exception:
        # transient NRT/axon failures: retry once
        time.sleep(2.0)
        res = run_bass_kernel_spmd(nc, in_maps, list(range(NCORES)))
    results = res.results

    all_t, all_c = [], []
    for c in range(NCORES):
        r = results[c]
        mask = np.asarray(r["mask"]).reshape(384, T_LOC)
        c8 = np.asarray(r["c8"]).reshape(384, T_LOC // 8)
        o_c = c * INT - starts[c]
        t_, c_ = _postprocess_core(mask, c8, traces, nbr,
                                   o_c, c * INT, INT, N)
        all_t.append(t_)
        all_c.append(c_)

    times = np.concatenate(all_t) if all_t else np.empty(0, np.int64)
    chans = np.concatenate(all_c) if all_c else np.empty(0, np.int64)
    times, chans = times[:MAX_DET], chans[:MAX_DET]
    out_t = np.full(MAX_DET, -1, np.int64)
    out_c = np.full(MAX_DET, -1, np.int32)
    out_t[: times.size] = times
    out_c[: chans.size] = chans
    return out_t, out_c
